# revision 10
# baseline (speedup 1.0000x reference)
"""Max-pooling over sequence spans — Trainium2 Bass kernel (v4).

Problem: context [B=8, S=4096, H=1024] f32; spans_begin/spans_len [B, 100] i32.
Output [B, 100, H] f32: out[b, n] = max over rows context[b, begin:begin+max(len,1)].

Sharding: pure data-parallel over the batch axis — one batch row per
NeuronCore, 8 cores, no cross-device communication.

v4 algorithm (per core), built around the SWDGE MoE primitives:
  * Host converts the context to bf16 (rel err ~4e-3, tolerance is 2e-2) and
    flattens all span rows into one position list: each span padded to a
    multiple of K=8 rows (dup of its last row — a no-op under max), split
    into K equal chips.  Chip t of every span lives in chip-block t of the
    list, so the whole reduction over chips is K-1 global tensor_tensor max
    ops.  Spans are sorted by padded size so the final per-span tails group
    into at most 8 equal-length runs.
  * One schedule is baked for all 8 cores (slot-wise max over the per-core
    sorted chip sizes); each core supplies its own gather indices.
  * Device: 8x dma_gather(transpose=True) pulls the rows feature-major
    ([128, 8, N8] bf16, dst[p,e,i] = row_i[e*128+p]) straight from DRAM —
    ~8 MiB instead of the 25.6 MiB the naive per-step gather reads, in 8
    instructions instead of 64 (SWDGE prep is ~1us serial per instruction).
  * DVE folds chip-blocks pairwise (tensor_tensor max in 2x bf16 mode),
    then <=8 grouped tensor_reduce ops collapse each span's remaining tail.
  * The [128, 8, 128] feature-major result is DMA'd out raw; the host
    un-transposes and un-sorts (free) and casts to f32.

kernel() compiles one program per baked schedule signature and caches it.
Any failure in the v4 path falls back to the v2 value-generic program.
"""

import sys
import numpy as np

sys.path.insert(0, "/opt/trn_rl_repo")

B, S, H = 8, 4096, 1024
N_SPANS = 100
MAX_LEN = 64
N_CORES = 8
K = 8            # chips per span
E = H // 128     # feature blocks

_cache = {}


# ---------------------------------------------------------------- v4 path

def _plan_schedule(begins, lens):
    """begins/lens: [B, N_SPANS] int arrays. One baked schedule for all cores."""
    eff = np.maximum(lens, 1)
    m8 = -(-eff // K)                                  # ceil(eff/K), in [1, 8]
    order = np.argsort(-m8, axis=1, kind="stable")     # [B, N] slot -> span id
    m8_sorted = np.take_along_axis(m8, order, axis=1)
    shat = m8_sorted.max(axis=0)                       # [N] baked, descending
    a = np.zeros(N_SPANS + 1, np.int64)
    a[1:] = np.cumsum(shat)
    n8 = int(a[N_SPANS])
    N8 = -(-n8 // 128) * 128
    N = K * N8
    # dma_gather rejects num_idxs > 896 per instruction (SWDGE ring bound)
    assert N8 <= 896, f"span load too large for v4 schedule: N8={N8}"


    begin_sorted = np.take_along_axis(begins, order, axis=1)
    eff_sorted = np.take_along_axis(eff, order, axis=1)

    # per-core position lists; positions [a[N_SPANS], N8) of each chip
    # block are -1: a trailing run per gather slice, skipped by the SWDGE
    # ucode (no read, no write) and never touched by the trimmed folds
    idx = np.full((B, N), -1, np.int16)
    sh_i = shat.astype(np.int64)
    for b in range(B):
        rows_all = []
        for s in range(N_SPANS):
            sh = int(sh_i[s])
            r = np.arange(K * sh)
            rows_all.append(np.clip(begin_sorted[b, s]
                            + np.minimum(r, eff_sorted[b, s] - 1), 0, S - 1))
        flat = np.concatenate(rows_all)                # [8 * n8] chip-major per span
        # reorder: chip t of span s -> position t*N8 + a[s]
        pos = np.empty(len(flat), np.int64)
        o = 0
        for s in range(N_SPANS):
            sh = int(sh_i[s])
            for t in range(K):
                pos[o + t * sh:o + (t + 1) * sh] = t * N8 + a[s] + np.arange(sh)
            o += K * sh
        idx[b, pos] = flat.astype(np.int16)

    idxs_t = np.zeros((B, 128, N // 16), np.int16)
    for b in range(B):
        w = idx[b].reshape(N // 16, 16).T
        idxs_t[b] = np.tile(w, (8, 1))

    runs = []
    s = 0
    while s < N_SPANS:
        e2 = s
        while e2 < N_SPANS and shat[e2] == shat[s]:
            e2 += 1
        runs.append((s, e2 - s, int(shat[s])))
        s = e2
    return dict(shat=tuple(int(x) for x in shat), a=a, N8=N8, N=N, n8v=n8,
                runs=runs, idxs_t=idxs_t, order=order)


def _build_v4(shat, N8, N, runs, a, n8v, repeat=1, queue_map=None):
    import concourse.bacc as bacc
    import concourse.mybir as mybir
    import concourse.tile as tile
    from concourse.library_config import mlp

    nc = bacc.Bacc("TRN2", target_bir_lowering=False, debug=False,
                   num_devices=N_CORES, num_swdge_queues=4)
    ctx_d = nc.dram_tensor("ctx", [S, H], mybir.dt.bfloat16, kind="ExternalInput")
    idx_d = nc.dram_tensor("idx", [128, N // 16], mybir.dt.int16,
                           kind="ExternalInput")
    out_d = nc.dram_tensor("out", [128, E * 128], mybir.dt.bfloat16,
                           kind="ExternalOutput")

    qi = [0]
    emitted = []
    with tile.TileContext(nc) as tc:
        with tc.tile_pool(name="p", bufs=1) as p:
            idx_t = p.tile([128, N // 16], mybir.dt.int16)
            nc.sync.dma_start(out=idx_t[:], in_=idx_d[:])
            nc.gpsimd.load_library(mlp)
            for _ in range(repeat):
                dsts = []
                for g in range(K):
                    dst_g = p.tile([128, E, N8], mybir.dt.bfloat16,
                                   tag=f"dst{g}", name=f"dst{g}")
                    dsts.append(dst_g)
                res2 = p.tile([128, E, 128], mybir.dt.bfloat16, tag="res2")
                nc.vector.memset(res2[:], 0.0)
                for g in range(K):
                    qn = (queue_map[qi[0]] if queue_map is not None
                          else g % 4)
                    qi[0] += 1
                    gi = nc.gpsimd.dma_gather(
                        dsts[g][:], ctx_d[:],
                        idx_t[:, g * N8 // 16:(g + 1) * N8 // 16],
                        N8, n8v, H, transpose=True, queue_num=qn)
                    emitted.append(gi)
                # adjacent-pair fold tree over valid columns only, so each
                # fold can start as soon as its two gathers land
                stride = 1
                while stride < K:
                    for t in range(0, K, 2 * stride):
                        nc.vector.tensor_tensor(
                            out=dsts[t][:, :, 0:n8v],
                            in0=dsts[t][:, :, 0:n8v],
                            in1=dsts[t + stride][:, :, 0:n8v],
                            op=mybir.AluOpType.max)
                    stride *= 2
                for (s0, kk, sh) in runs:
                    base = int(a[s0])
                    in_ap = dsts[0][:, :, base:base + kk * sh]
                    in_ap = in_ap.rearrange("p e (k s) -> p k e s", k=kk)
                    out_ap = res2[:, :, s0:s0 + kk].rearrange("p e k -> p k e")
                    if sh == 1:
                        nc.vector.tensor_copy(out=out_ap, in_=in_ap.squeeze(3))
                    else:
                        nc.vector.tensor_reduce(
                            out=out_ap, in_=in_ap,
                            axis=mybir.AxisListType.X, op=mybir.AluOpType.max)
                nc.sync.dma_start(
                    out=out_d[:],
                    in_=res2[:].rearrange("p e k -> p (e k)"))
    nc.compile()
    nc.__dict__["_v4_gathers"] = emitted
    return nc


def _gather_lanes(nc):
    """Scheduled DMASW lane per dma_gather, in emission order."""
    from concourse.tile_sem_assignment import PROC_NAME_TO_IDX
    idx_to_proc = {v: k for k, v in PROC_NAME_TO_IDX.items()}
    lanes = []
    import re
    for bi in nc.__dict__["_v4_gathers"]:
        m = re.search(r"update:S\[DMASW(\d+)_", str(bi))
        lanes.append(int(m.group(1)))
    return lanes


def _get_v4(plan, repeat=1):
    key = ("v4", plan["shat"], repeat)
    if key not in _cache:
        args = (plan["shat"], plan["N8"], plan["N"], plan["runs"], plan["a"],
                plan["n8v"])
        nc = _build_v4(*args, repeat=repeat)
        if repeat > 1:
            # timing builds interleave repeats; realign each gather's SWDGE
            # queue with its scheduled completion-sem lane (lane %% 4) so no
            # lane is updated from two queues
            lanes = _gather_lanes(nc)
            nc = _build_v4(*args, repeat=repeat,
                           queue_map=[l % 4 for l in lanes])
        _cache[key] = nc
    return _cache[key]


def _unscramble(out_raw, order_b):
    r = np.asarray(out_raw).reshape(128, E, 128)
    vals = r.transpose(2, 1, 0).reshape(128, E * 128)
    out = np.empty((N_SPANS, H), np.float32)
    out[order_b] = vals[:N_SPANS].astype(np.float32)
    return out


def _kernel_v4(context, spans_begin, spans_len):
    import ml_dtypes
    from concourse.bass_utils import run_bass_kernel_spmd

    plan = _plan_schedule(spans_begin.astype(np.int64),
                          spans_len.astype(np.int64))
    nc = _get_v4(plan)
    ctx_bf = context.astype(ml_dtypes.bfloat16)
    in_maps = [{"ctx": ctx_bf[b], "idx": plan["idxs_t"][b]} for b in range(B)]
    res = run_bass_kernel_spmd(nc, in_maps, list(range(N_CORES)))
    out = np.empty((B, N_SPANS, H), np.float32)
    for b in range(B):
        out[b] = _unscramble(res.results[b]["out"], plan["order"][b])
    return out


# ------------------------------------------------- v2 fallback (value-generic)

def _build_program(n_steps, k_bufs, repeat=1, n_slab_bufs=6):
    """Per-step indirect-gather + DVE max chain (the previous baseline)."""
    import concourse.bass as bass
    import concourse.bacc as bacc
    import concourse.mybir as mybir
    import concourse.tile as tile

    nc = bacc.Bacc("TRN2", target_bir_lowering=False, debug=False,
                   num_devices=N_CORES)
    ctx_d = nc.dram_tensor("ctx", [S, H], mybir.dt.float32, kind="ExternalInput")
    idx_d = nc.dram_tensor("idx", [N_SPANS, n_steps], mybir.dt.int32,
                           kind="ExternalInput")
    out_d = nc.dram_tensor("out", [N_SPANS, H], mybir.dt.float32,
                           kind="ExternalOutput")

    with tile.TileContext(nc) as tc:
        with (
            tc.tile_pool(name="persist", bufs=1) as persist,
            tc.tile_pool(name="slabs", bufs=n_slab_bufs) as slabs,
        ):
            idx_t = persist.tile([N_SPANS, n_steps], mybir.dt.int32)
            nc.sync.dma_start(out=idx_t[:], in_=idx_d[:])
            for _ in range(repeat):
                accs = []
                for k in range(k_bufs):
                    acc = persist.tile([N_SPANS, H], mybir.dt.float32,
                                       tag=f"acc{k}")
                    nc.vector.memset(acc[:], -3.0e38)
                    accs.append(acc)
                for l in range(n_steps):
                    slab = slabs.tile([N_SPANS, H], mybir.dt.float32)
                    nc.gpsimd.indirect_dma_start(
                        out=slab[:],
                        out_offset=None,
                        in_=ctx_d[:],
                        in_offset=bass.IndirectOffsetOnAxis(
                            ap=idx_t[:, l:l + 1], axis=0),
                    )
                    acc = accs[l % k_bufs]
                    nc.vector.tensor_tensor(out=acc[:], in0=acc[:],
                                            in1=slab[:],
                                            op=mybir.AluOpType.max)
                step = 1
                while step < k_bufs:
                    for k in range(0, k_bufs, 2 * step):
                        if k + step < k_bufs:
                            nc.vector.tensor_tensor(
                                out=accs[k][:], in0=accs[k][:],
                                in1=accs[k + step][:],
                                op=mybir.AluOpType.max)
                    step *= 2
                nc.sync.dma_start(out=out_d[:], in_=accs[0][:])
    nc.compile()
    return nc


def _get_program(n_steps=MAX_LEN):
    key = ("v1", n_steps, 2)
    if key not in _cache:
        _cache[key] = _build_program(n_steps, 2)
    return _cache[key]


def _make_indices(spans_begin, spans_len, n_steps=MAX_LEN):
    eff = np.maximum(spans_len, 1)
    steps = np.arange(n_steps, dtype=np.int32)
    idx = spans_begin[:, :, None] + np.minimum(steps[None, None, :],
                                               eff[:, :, None] - 1)
    return np.clip(idx, 0, S - 1).astype(np.int32)


def _kernel_v2(context, spans_begin, spans_len):
    from concourse.bass_utils import run_bass_kernel_spmd

    n_steps = int(min(MAX_LEN, max(1, np.maximum(spans_len, 1).max())))
    idx = _make_indices(spans_begin, spans_len, n_steps)
    nc = _get_program(n_steps)
    in_maps = [{"ctx": context[b], "idx": idx[b]} for b in range(B)]
    res = run_bass_kernel_spmd(nc, in_maps, list(range(N_CORES)))
    out = np.stack([res.results[b]["out"] for b in range(B)], axis=0)
    return out.astype(np.float32)


def kernel(context, spans_begin, spans_len):
    context = np.ascontiguousarray(context, dtype=np.float32)
    spans_begin = np.asarray(spans_begin, dtype=np.int32)
    spans_len = np.asarray(spans_len, dtype=np.int32)
    assert context.shape == (B, S, H), context.shape
    assert spans_begin.shape == (B, N_SPANS), spans_begin.shape

    try:
        return _kernel_v4(context, spans_begin, spans_len)
    except Exception:
        import traceback
        traceback.print_exc()
        return _kernel_v2(context, spans_begin, spans_len)
